# revision 11
# baseline (speedup 1.0000x reference)
"""Trainium2 Bass kernel for nn_CorrBlock: softmax(fmap1 @ fmap2.T / sqrt(D), axis=-1).

Sharding: fmap1 rows split across 8 cores (1024 rows each), fmap2 replicated.
Each core computes its [1024, 8192] slab of the output independently.

Device kernel (per core):
  - Inputs are pre-transposed on the host so the contraction dim lands on SBUF
    partitions with no on-device transpose, and f2 is chunk-major so each
    2048-col chunk is one contiguous 8KB line per partition (128 DMA
    descriptors per chunk — the DMA rings are descriptor-rate limited at
    ~200ns/descriptor, so line size matters more than bytes).
  - PE: matmuls accumulate the D=256 contraction in 2 chunks of 128 into PSUM.
  - ACT: Exp with fused 1/sqrt(D) scale reads PSUM, writes SBUF (fp16), and
    emits per-row partial sums via accum_out in the same pass.
  - DVE: reciprocal of the row sum, then per-row scalar multiply (4x fp16 mode).
  - DMA out the normalized [128, 8192] block as fp16; host upcasts to fp32.

The ACT engine is the bottleneck (exp of 8.4M elements/core at ~1.2G elem/s/
partition; ~1.9us busy per 2048-col chunk, cost = 2048 cycles + SBUF access
init — both confirmed against the instruction cost model and the trace). The
wavefront schedule (chunk-0 EXPs of the first LEAD blocks run first) buys time
for the later f2 chunks to arrive while keeping ACT gapless from ~5us on, and
spreads each block's DVE normalize + output DMA evenly through the run.
"""

import os
import sys

import numpy as np

if "/opt/trn_rl_repo" not in sys.path:
    sys.path.insert(0, "/opt/trn_rl_repo")

import concourse.bacc as bacc
import concourse.bass as bass
import concourse.mybir as mybir
import concourse.tile as tile
from concourse.bass_utils import run_bass_kernel_spmd

N, M, D = 8192, 8192, 256
N_CORES = 8
NB = N // N_CORES  # rows per core
DC = D // 128  # contraction chunks
QC = 2048  # columns per PSUM tile (4 banks); 2 in flight ping-pong
NQ = M // QC

MM_DT = os.environ.get("CORR_MM_DT", "float16")
OUT_DT = os.environ.get("CORR_OUT_DT", "float16")
LEAD = int(os.environ.get("CORR_LEAD", "3"))  # blocks that run chunk-0 first
ACCUM = os.environ.get("CORR_ACCUM", "act")  # act: accum_out; dve: reduce_sum

# Populated by kernel() on every run (exec_time_ns only when tracing).
last_run_info: dict = {}


def build_nc(nb=NB, m=M, dc=DC, qc=QC, mm_dt=None, out_dt=None, exp_bufs=4):
    f32 = mybir.dt.float32
    mm_dtype = getattr(mybir.dt, mm_dt or MM_DT)
    out_dtype = getattr(mybir.dt, out_dt or OUT_DT)
    n_blocks = nb // 128
    n_q = m // qc
    scale = 1.0 / (D**0.5)

    nc = bacc.Bacc("TRN2", target_bir_lowering=False, debug=False)

    f1t = nc.dram_tensor("f1t", [128, dc, nb], mm_dtype, kind="ExternalInput")
    # chunk-major: [partition, chunk, dc, col-in-chunk]
    f2t = nc.dram_tensor("f2t", [128, n_q, dc, qc], mm_dtype, kind="ExternalInput")
    out = nc.dram_tensor("out", [nb, m], out_dtype, kind="ExternalOutput")

    # EXP issue order: chunk-0 for the first LEAD blocks, then per-block
    # chunk-major with the lead blocks finishing their remaining chunks first.
    sched = [(b, 0) for b in range(LEAD)]
    for b in range(LEAD):
        sched += [(b, q) for q in range(1, n_q)]
    for b in range(LEAD, n_blocks):
        sched += [(b, q) for q in range(n_q)]
    assert len(sched) == n_blocks * n_q

    with tile.TileContext(nc) as tc:
        with (
            tc.tile_pool(name="weights", bufs=1) as wpool,
            tc.tile_pool(name="exps", bufs=exp_bufs) as epool,
            tc.tile_pool(name="stats", bufs=4) as spool,
            tc.tile_pool(name="psum", bufs=2, space="PSUM") as ppool,
        ):
            # Input DMAs, priority order. The rings drain descriptors FIFO in
            # issue order, so f1 + chunk 0 complete first; ring throughput
            # ramps over the first ~8us, which (after the ~5.5us framework
            # preamble) puts chunk 0 on SBUF at ~13us no matter how the
            # transfers are sliced.
            f1s = wpool.tile([128, dc, nb], mm_dtype, tag="f1s")
            nc.sync.dma_start(f1s[:], f1t[:])
            f2s = []
            for q in range(n_q):
                f2q = wpool.tile([128, dc, qc], mm_dtype, tag=f"f2q_{q}", name=f"f2q_{q}")
                nc.sync.dma_start(f2q[:], f2t[:, q])
                f2s.append(f2q)

            # While the inputs stream in: load the Exp table on ACT (saves
            # ~1.3us on the critical path) and run warmup matmuls so the PE
            # clock is ramped when the first real chunk arrives (cold matmuls
            # run at half speed). Scratch data, never read.
            wm = wpool.tile([128, 640], mm_dtype, tag="warm")
            tld = wpool.tile([128, 2], f32, tag="tld")
            nc.gpsimd.memset(wm[:], 0)
            nc.scalar.activation(
                tld[:], wm[:, 0:2], mybir.ActivationFunctionType.Exp, scale=scale
            )
            psw = ppool.tile([128, 4, 512], f32, tag="ps", name="ps_warm")
            for _ in range(13):
                nc.tensor.matmul(
                    psw[:, 0, :], wm[:, 0:128], wm[:, 128:640], start=True, stop=True
                )

            exps = {}
            sums = {}

            def do_exp(b, q, ps_ap, col, lo, hi):
                esl = exps[b][:, lo:hi]
                if ACCUM == "act":
                    nc.scalar.activation(
                        esl,
                        ps_ap,
                        mybir.ActivationFunctionType.Exp,
                        scale=scale,
                        accum_out=sums[b][:, col : col + 1],
                    )
                else:
                    nc.scalar.activation(
                        esl, ps_ap, mybir.ActivationFunctionType.Exp, scale=scale
                    )
                    nc.vector.reduce_sum(
                        sums[b][:, col : col + 1], esl, axis=mybir.AxisListType.X
                    )

            for i, (b, q) in enumerate(sched):
                if b not in exps:
                    exps[b] = epool.tile([128, m], out_dtype, tag="exps", name=f"exps_{b}")
                    # one spare partial-sum column for the split first chunk
                    sums[b] = spool.tile([128, n_q + 1], f32, tag="sums", name=f"sums_{b}")
                n_j = qc // 512
                ps = ppool.tile([128, n_j, 512], f32, tag="ps", name=f"ps_{b}_{q}")
                for d in range(dc):
                    lhsT = f1s[:, d, b * 128 : (b + 1) * 128]
                    for j in range(n_j):
                        nc.tensor.matmul(
                            ps[:, j, :],
                            lhsT,
                            f2s[q][:, d, j * 512 : (j + 1) * 512],
                            start=(d == 0),
                            stop=(d == dc - 1),
                        )
                if i == 0:
                    # Split the first EXP in half so the ACT stream starts as
                    # soon as the first two matmuls finish.
                    half = qc // 2
                    do_exp(b, q, ps[:, 0 : n_j // 2].rearrange("p a b -> p (a b)"),
                           q, q * qc, q * qc + half)
                    do_exp(b, q, ps[:, n_j // 2 :].rearrange("p a b -> p (a b)"),
                           n_q, q * qc + half, (q + 1) * qc)
                    continue
                do_exp(b, q, ps.rearrange("p a b -> p (a b)"), q,
                       q * qc, (q + 1) * qc)
                if q == n_q - 1:
                    rsum = spool.tile([128, 1], f32, tag="rsum", name=f"rsum_{b}")
                    recip = spool.tile([128, 1], f32, tag="recip", name=f"recip_{b}")
                    ncols = n_q + 1 if b == 0 else n_q
                    nc.vector.reduce_sum(
                        rsum[:], sums[b][:, 0:ncols], axis=mybir.AxisListType.X
                    )
                    nc.vector.reciprocal(recip[:], rsum[:])
                    if b < n_blocks - 1:
                        # One whole-block output DMA: 16KB DRAM lines, 4x
                        # fewer descriptors (the rings are desc-rate bound).
                        for qq in range(n_q):
                            sl = slice(qq * qc, (qq + 1) * qc)
                            nc.vector.tensor_scalar_mul(
                                exps[b][:, sl], exps[b][:, sl], recip[:]
                            )
                        nc.sync.dma_start(
                            out[b * 128 : (b + 1) * 128, :], exps[b][:]
                        )
                    else:
                        # Last block: per-chunk mul+DMA so the drain pipelines
                        # behind the multiplies.
                        for qq in range(n_q):
                            sl = slice(qq * qc, (qq + 1) * qc)
                            nc.vector.tensor_scalar_mul(
                                exps[b][:, sl], exps[b][:, sl], recip[:]
                            )
                            nc.sync.dma_start(
                                out[b * 128 : (b + 1) * 128, sl], exps[b][:, sl]
                            )

    nc.compile()
    return nc


_nc_cache: dict = {}


def _get_nc():
    key = (MM_DT, OUT_DT, LEAD, ACCUM)
    if key not in _nc_cache:
        _nc_cache[key] = build_nc()
    return _nc_cache[key]


def kernel(fmap1: np.ndarray, fmap2: np.ndarray) -> np.ndarray:
    f1 = np.asarray(fmap1, dtype=np.float32)
    f2 = np.asarray(fmap2, dtype=np.float32)
    np_mm = mybir.dt.np(getattr(mybir.dt, MM_DT))
    # [rows, D] -> [128, D/128, rows]: f1t[dp, dcc, n] = f1[n, dcc*128 + dp]
    f1t = np.ascontiguousarray(
        f1.T.reshape(DC, 128, N).transpose(1, 0, 2).astype(np_mm)
    )
    # [rows, D] -> [128, NQ, D/128, QC]: f2t[dp, q, dcc, c] = f2[q*QC+c, dcc*128+dp]
    f2t = np.ascontiguousarray(
        f2.T.reshape(DC, 128, NQ, QC).transpose(1, 2, 0, 3).astype(np_mm)
    )

    nc = _get_nc()
    in_maps = [
        {"f1t": np.ascontiguousarray(f1t[:, :, i * NB : (i + 1) * NB]), "f2t": f2t}
        for i in range(N_CORES)
    ]
    trace = bool(os.environ.get("BASS_TRACE"))
    res = run_bass_kernel_spmd(nc, in_maps, list(range(N_CORES)), trace=trace)
    last_run_info.clear()
    last_run_info.update(
        exec_time_ns=res.exec_time_ns,
        mean_exec_time_ns=res.mean_exec_time_ns,
        profile_json=res.profile_json,
        trace_path=(res.instructions_and_trace or (None, None))[1],
    )
    return np.concatenate(
        [res.results[i]["out"] for i in range(N_CORES)], axis=0
    ).astype(np.float32)


# revision 13
# speedup vs baseline: 1.2993x; 1.2993x over previous
"""Trainium2 Bass kernel for nn_CorrBlock: softmax(fmap1 @ fmap2.T / sqrt(D), axis=-1).

Sharding: fmap1 rows split across 8 cores (1024 rows each), fmap2 replicated.
Each core computes its [1024, 8192] slab of the output independently.

Device kernel (per core):
  - Inputs are pre-transposed on the host so the contraction dim lands on SBUF
    partitions with no on-device transpose, and f2 is chunk-major so each
    2048-col chunk is one contiguous 8KB line per partition (128 DMA
    descriptors per chunk — the DMA rings are descriptor-rate limited at
    ~200ns/descriptor, so line size matters more than bytes).
  - PE: matmuls accumulate the D=256 contraction in 2 chunks of 128 into PSUM.
  - ACT: Exp with fused 1/sqrt(D) scale reads PSUM, writes SBUF (fp16), and
    emits per-row partial sums via accum_out in the same pass.
  - DVE: reciprocal of the row sum, then per-row scalar multiply (4x fp16 mode).
  - DMA out the normalized [128, 8192] block as fp16; host upcasts to fp32.

The ACT engine is the bottleneck (exp of 8.4M elements/core at ~1.2G elem/s/
partition; ~1.9us busy per 2048-col chunk, cost = 2048 cycles + SBUF access
init — both confirmed against the instruction cost model and the trace). The
wavefront schedule (chunk-0 EXPs of the first LEAD blocks run first) buys time
for the later f2 chunks to arrive while keeping ACT gapless from ~5us on, and
spreads each block's DVE normalize + output DMA evenly through the run.
"""

import os
import sys

import numpy as np

if "/opt/trn_rl_repo" not in sys.path:
    sys.path.insert(0, "/opt/trn_rl_repo")

import concourse.bacc as bacc
import concourse.bass as bass
import concourse.mybir as mybir
import concourse.tile as tile
from concourse.bass_utils import run_bass_kernel_spmd

N, M, D = 8192, 8192, 256
N_CORES = 8
NB = N // N_CORES  # rows per core
DC = D // 128  # contraction chunks
QC = 2048  # columns per PSUM tile (4 banks); 2 in flight ping-pong
NQ = M // QC

MM_DT = os.environ.get("CORR_MM_DT", "float16")
OUT_DT = os.environ.get("CORR_OUT_DT", "float16")
LEAD = int(os.environ.get("CORR_LEAD", "3"))  # blocks that run chunk-0 first
ACCUM = os.environ.get("CORR_ACCUM", "act")  # act: accum_out; dve: reduce_sum

# Populated by kernel() on every run (exec_time_ns only when tracing).
last_run_info: dict = {}


def build_nc(nb=NB, m=M, dc=DC, qc=QC, mm_dt=None, out_dt=None, exp_bufs=4):
    f32 = mybir.dt.float32
    mm_dtype = getattr(mybir.dt, mm_dt or MM_DT)
    out_dtype = getattr(mybir.dt, out_dt or OUT_DT)
    n_blocks = nb // 128
    n_q = m // qc
    scale = 1.0 / (D**0.5)

    nc = bacc.Bacc("TRN2", target_bir_lowering=False, debug=False)

    f1t = nc.dram_tensor("f1t", [128, dc, nb], mm_dtype, kind="ExternalInput")
    # chunk-major: [partition, chunk, dc, col-in-chunk]
    f2t = nc.dram_tensor("f2t", [128, n_q, dc, qc], mm_dtype, kind="ExternalInput")
    out = nc.dram_tensor("out", [nb, m], out_dtype, kind="ExternalOutput")

    # EXP issue order: chunk-0 for the first LEAD blocks, then per-block
    # chunk-major with the lead blocks finishing their remaining chunks first.
    sched = [(b, 0) for b in range(LEAD)]
    for b in range(LEAD):
        sched += [(b, q) for q in range(1, n_q)]
    for b in range(LEAD, n_blocks):
        sched += [(b, q) for q in range(n_q)]
    assert len(sched) == n_blocks * n_q

    with tile.TileContext(nc) as tc:
        with (
            tc.tile_pool(name="weights", bufs=1) as wpool,
            tc.tile_pool(name="exps", bufs=exp_bufs) as epool,
            tc.tile_pool(name="stats", bufs=4) as spool,
            tc.tile_pool(name="psum", bufs=2, space="PSUM") as ppool,
        ):
            # Input DMAs, priority order. The rings drain descriptors FIFO in
            # issue order, so f1 + chunk 0 complete first; ring throughput
            # ramps over the first ~8us, which (after the ~5.5us framework
            # preamble) puts chunk 0 on SBUF at ~13us no matter how the
            # transfers are sliced.
            f1s = wpool.tile([128, dc, nb], mm_dtype, tag="f1s")
            nc.sync.dma_start(f1s[:], f1t[:])
            f2s = []
            for q in range(n_q):
                f2q = wpool.tile([128, dc, qc], mm_dtype, tag=f"f2q_{q}", name=f"f2q_{q}")
                nc.sync.dma_start(f2q[:], f2t[:, q])
                f2s.append(f2q)

            # While the inputs stream in: load the Exp table on ACT (saves
            # ~1.3us on the critical path) and run warmup matmuls so the PE
            # clock is ramped when the first real chunk arrives (cold matmuls
            # run at half speed). Scratch data, never read.
            wm = wpool.tile([128, 640], mm_dtype, tag="warm")
            tld = wpool.tile([128, 2], f32, tag="tld")
            nc.gpsimd.memset(wm[:], 0)
            nc.scalar.activation(
                tld[:], wm[:, 0:2], mybir.ActivationFunctionType.Exp, scale=scale
            )
            psw = ppool.tile([128, 4, 512], f32, tag="ps", name="ps_warm")
            for _ in range(13):
                nc.tensor.matmul(
                    psw[:, 0, :], wm[:, 0:128], wm[:, 128:640], start=True, stop=True
                )

            exps = {}
            sums = {}

            def do_exp(b, q, ps_ap, col, lo, hi):
                esl = exps[b][:, lo:hi]
                if ACCUM == "act":
                    nc.scalar.activation(
                        esl,
                        ps_ap,
                        mybir.ActivationFunctionType.Exp,
                        scale=scale,
                        accum_out=sums[b][:, col : col + 1],
                    )
                elif ACCUM == "ts":
                    # Row sums via an in-place x1.0 tensor_scalar on DVE
                    # (fast-mode, ~0.7us/chunk) instead of ACT's accumulator
                    # read (0.18us/chunk of bottleneck-engine time).
                    nc.scalar.activation(
                        esl, ps_ap, mybir.ActivationFunctionType.Exp, scale=scale
                    )
                    nc.vector.tensor_scalar(
                        esl,
                        esl,
                        1.0,
                        None,
                        mybir.AluOpType.mult,
                        accum_out=sums[b][:, col : col + 1],
                    )
                else:
                    nc.scalar.activation(
                        esl, ps_ap, mybir.ActivationFunctionType.Exp, scale=scale
                    )
                    nc.vector.reduce_sum(
                        sums[b][:, col : col + 1], esl, axis=mybir.AxisListType.X
                    )

            for i, (b, q) in enumerate(sched):
                if b not in exps:
                    exps[b] = epool.tile([128, m], out_dtype, tag="exps", name=f"exps_{b}")
                    # one spare partial-sum column for the split first chunk
                    sums[b] = spool.tile([128, n_q + 1], f32, tag="sums", name=f"sums_{b}")
                n_j = qc // 512
                ps = ppool.tile([128, n_j, 512], f32, tag="ps", name=f"ps_{b}_{q}")
                for d in range(dc):
                    lhsT = f1s[:, d, b * 128 : (b + 1) * 128]
                    for j in range(n_j):
                        nc.tensor.matmul(
                            ps[:, j, :],
                            lhsT,
                            f2s[q][:, d, j * 512 : (j + 1) * 512],
                            start=(d == 0),
                            stop=(d == dc - 1),
                        )
                if i == 0:
                    # Split the first EXP in half so the ACT stream starts as
                    # soon as the first two matmuls finish.
                    half = qc // 2
                    do_exp(b, q, ps[:, 0 : n_j // 2].rearrange("p a b -> p (a b)"),
                           q, q * qc, q * qc + half)
                    do_exp(b, q, ps[:, n_j // 2 :].rearrange("p a b -> p (a b)"),
                           n_q, q * qc + half, (q + 1) * qc)
                    continue
                do_exp(b, q, ps.rearrange("p a b -> p (a b)"), q,
                       q * qc, (q + 1) * qc)
                if q == n_q - 1:
                    rsum = spool.tile([128, 1], f32, tag="rsum", name=f"rsum_{b}")
                    recip = spool.tile([128, 1], f32, tag="recip", name=f"recip_{b}")
                    ncols = n_q + 1 if b == 0 else n_q
                    nc.vector.reduce_sum(
                        rsum[:], sums[b][:, 0:ncols], axis=mybir.AxisListType.X
                    )
                    nc.vector.reciprocal(recip[:], rsum[:])
                    if b < n_blocks - 1:
                        # One whole-block output DMA: 16KB DRAM lines, 4x
                        # fewer descriptors (the rings are desc-rate bound).
                        for qq in range(n_q):
                            sl = slice(qq * qc, (qq + 1) * qc)
                            nc.vector.tensor_scalar_mul(
                                exps[b][:, sl], exps[b][:, sl], recip[:]
                            )
                        nc.sync.dma_start(
                            out[b * 128 : (b + 1) * 128, :], exps[b][:]
                        )
                    else:
                        # Last block: fine mul pieces so the first output DMA
                        # starts ASAP; the 2MB drain is byte-bound after the
                        # final EXP, so every ns earlier helps.
                        fine = 1024
                        for p in range(m // fine):
                            sl = slice(p * fine, (p + 1) * fine)
                            nc.vector.tensor_scalar_mul(
                                exps[b][:, sl], exps[b][:, sl], recip[:]
                            )
                            nc.sync.dma_start(
                                out[b * 128 : (b + 1) * 128, sl], exps[b][:, sl]
                            )

    nc.compile()
    return nc


_nc_cache: dict = {}


def _get_nc():
    key = (MM_DT, OUT_DT, LEAD, ACCUM)
    if key not in _nc_cache:
        _nc_cache[key] = build_nc()
    return _nc_cache[key]


def kernel(fmap1: np.ndarray, fmap2: np.ndarray) -> np.ndarray:
    f1 = np.asarray(fmap1, dtype=np.float32)
    f2 = np.asarray(fmap2, dtype=np.float32)
    np_mm = mybir.dt.np(getattr(mybir.dt, MM_DT))
    # [rows, D] -> [128, D/128, rows]: f1t[dp, dcc, n] = f1[n, dcc*128 + dp]
    f1t = np.ascontiguousarray(
        f1.T.reshape(DC, 128, N).transpose(1, 0, 2).astype(np_mm)
    )
    # [rows, D] -> [128, NQ, D/128, QC]: f2t[dp, q, dcc, c] = f2[q*QC+c, dcc*128+dp]
    f2t = np.ascontiguousarray(
        f2.T.reshape(DC, 128, NQ, QC).transpose(1, 2, 0, 3).astype(np_mm)
    )

    nc = _get_nc()
    in_maps = [
        {"f1t": np.ascontiguousarray(f1t[:, :, i * NB : (i + 1) * NB]), "f2t": f2t}
        for i in range(N_CORES)
    ]
    trace = bool(os.environ.get("BASS_TRACE"))
    res = run_bass_kernel_spmd(nc, in_maps, list(range(N_CORES)), trace=trace)
    last_run_info.clear()
    last_run_info.update(
        exec_time_ns=res.exec_time_ns,
        mean_exec_time_ns=res.mean_exec_time_ns,
        profile_json=res.profile_json,
        trace_path=(res.instructions_and_trace or (None, None))[1],
    )
    return np.concatenate(
        [res.results[i]["out"] for i in range(N_CORES)], axis=0
    ).astype(np.float32)
